# revision 44
# baseline (speedup 1.0000x reference)
"""Trainium2 Bass kernel for nn_GCNDiehlq1 (SAGEConv x2 + lin + EdgePooling, x3
levels, mean readout + MLP + log_softmax).

Structure (validated against the jax reference at ~1e-7 rel err):
- All edges are within-graph, so SAGE mean aggregation == per-graph dense
  A_norm @ x with AT[s,d] = count(s->d)/max(indeg(d),1). The 64 graphs are
  sharded 8-per-core across 8 NeuronCores.
- The device kernel computes one level (2 SAGE convs + lin + edge-score
  projections u,v) for 8 graphs in feature-major layout. The builder is
  parameterized by slots-per-graph (320 for level 1; levels 2/3 pick
  256/128 variants based on actual post-pool graph sizes) -> one NEFF per
  size variant, reused across calls.
- Host does the inherently sequential EdgePooling (per-dst softmax from
  u,v, stable sort, greedy merge scan, coalesce) and the final readout.
  Per-graph compact cluster relabeling is strictly order-preserving
  w.r.t. the reference's global labels within each graph, so coalesce
  order and sort tie-breaks match the reference exactly.
"""

import numpy as np

N = 20480
NPG = 320
G = 64
F = 128
H = 128
C = 6
PADMAX = 384                 # host-side cluster-id padding (>= 320)
NCORES = 8
GPC = G // NCORES            # 8 graphs per core
NCHUNK = 512                 # dense matmul moving free dim

_compiled = {}


def _ktiles(sg):
    """K-tile (offset, length) list for sg slots per graph."""
    out = []
    off = 0
    while off < sg:
        out.append((off, min(128, sg - off)))
        off += 128
    return out


# ---------------------------------------------------------------- device ---

def _apply_tile_patch():
    """This walrus build rejects >1 sem waits on TPB_CTRL (Drain/NoOp):
    'Too many sync wait commands'. Split the TileContext exit-barrier waits
    across one NOP per logical proc, then emit the drain bare."""
    import concourse.tile as tile
    from concourse.vector_clock import ScopedClock, VectorClock

    if getattr(tile.TileContext, "_drain_patched", False):
        return

    def _patched(self, tick_clock, wait_clock):
        full = tick_clock.global_clock
        nprocs = len(full)
        for proc in range(nprocs):
            tick = full[proc]
            if tick <= 0:
                continue
            vec = [0] * nprocs
            vec[proc] = tick
            nop_inst = self.nc.sync.nop(nofuse=True, hint="pre_drain_%d" % proc)
            wait_clock.add_sem_waits(
                nop_inst.ins, ScopedClock({None: VectorClock(vec)})
            )
        self.nc.sync.drain()
        if getattr(type(self), "_keep_tail_barrier", True):
            self.nc.all_engine_barrier()
        assert self.sems is not None
        popped = self.nc._tile_sem_poison_stack.pop()
        assert popped is self._sem_poison
        if getattr(type(self), "_keep_sem_reset", True):
            self.nc.clear_and_free_semaphores(
                list(self.sems.allocated().values()))
            self.nc.all_engine_barrier()
        else:
            # skip the end-of-NEFF sem clear + second barrier; only do the
            # python-side free-pool bookkeeping
            sem_nums = [s.num for s in self.sems.allocated().values()]
            self.nc._state.prepend_free_semaphores(sem_nums)
            for poison_set in self.nc._tile_sem_poison_stack:
                poison_set.update(sem_nums)

    tile.TileContext._drain_and_barrier = _patched
    tile.TileContext._drain_patched = True
    # re-execution works without the end-of-NEFF sem clear: the runtime
    # reinitializes semaphore state per execution (verified empirically,
    # two back-to-back kernel() calls bitwise identical)
    tile.TileContext._keep_sem_reset = False
    tile.TileContext._keep_tail_barrier = False


def _split_multi_waits(nc):
    """This walrus build allows at most ONE sync wait per instruction.
    Insert single-wait NoOps (same engine, just before) for the extras."""
    import concourse.mybir as mybir

    for f in nc.m.functions:
        for bb in f.blocks:
            insts = list(bb.instructions)
            out = []
            changed = False
            for ins in insts:
                si = ins.sync_info
                if si is not None and len(si.on_wait) > 1:
                    waits = list(si.on_wait)
                    for j, w in enumerate(waits[:-1]):
                        nop = mybir.InstNoOp(name="%s_w%d" % (ins.name, j))
                        nop.engine = ins.engine
                        nop.sync_info = mybir.SyncInfo(on_wait=[w], on_update=[])
                        out.append(nop)
                    ins.sync_info = mybir.SyncInfo(
                        on_wait=[waits[-1]], on_update=list(si.on_update)
                    )
                    changed = True
                out.append(ins)
            if changed:
                bb.instructions = out


def _build_level_nc(sg, use_bf16):
    """One level for 8 graphs with sg slots per graph, feature-major
    activations [128, 8*sg]."""
    import concourse.bass as bass
    import concourse.mybir as mybir
    import concourse.tile as tile
    from concourse.masks import make_identity

    _apply_tile_patch()
    f32 = mybir.dt.float32
    f32r_mm = use_bf16 == "f32r"
    cdt = mybir.dt.bfloat16 if use_bf16 == "bf16" else f32
    mdt = mybir.dt.float32r if f32r_mm else cdt

    def mm(x):
        # reinterpret fp32 operands as float32r at matmul sites only
        return x.bitcast(mdt) if f32r_mm else x

    AF = mybir.ActivationFunctionType

    kts = _ktiles(sg)
    nk = len(kts)
    npc = GPC * sg
    nch = (npc + NCHUNK - 1) // NCHUNK
    assert npc % NCHUNK == 0

    nc = bass.Bass("TRN2", target_bir_lowering=False,
                   disable_frame_to_traceback=True)
    xT_d = nc.declare_dram_parameter("xT", [128, npc], cdt, isOutput=False)
    at_d = nc.declare_dram_parameter("AT", [128, GPC * nk, sg], cdt,
                                     isOutput=False)
    # all six [128,128] weights + w12 packed into one [128, 770] slab
    wpack_d = nc.declare_dram_parameter("wpack", [128, 6 * 128 + 2], cdt,
                                        isOutput=False)
    bpack_d = nc.declare_dram_parameter("bpack", [128, 3], f32, isOutput=False)
    hout_d = nc.declare_dram_parameter("houtT", [128, npc], cdt, isOutput=True)
    uv_d = nc.declare_dram_parameter("uv", [2, npc], f32, isOutput=True)

    with tile.TileContext(nc) as tc:
        with (
            tc.tile_pool(name="slab", bufs=1) as slab,
            tc.tile_pool(name="wts", bufs=1) as wts,
            tc.tile_pool(name="ps_agg", bufs=2, space="PSUM") as ps_agg,
            tc.tile_pool(name="ps_d", bufs=2, space="PSUM") as ps_d,
            tc.tile_pool(name="ps_tp", bufs=2, space="PSUM") as ps_tp,
            tc.tile_pool(name="ps_uv", bufs=2, space="PSUM") as ps_uv,
        ):
            ident = wts.tile([128, 128], cdt)
            make_identity(nc, ident[:])

            # weights via gpsimd (SWDGE) to keep the SP queue free for inputs
            wpack = wts.tile([128, 6 * 128 + 2], cdt)
            nc.gpsimd.dma_start(wpack[:], wpack_d[:])
            bpack = wts.tile([128, 3], f32)
            nc.gpsimd.dma_start(bpack[:], bpack_d[:])
            wnames = ("WL1", "WR1", "WL2", "WR2", "WLA", "WLB")
            W = {nm: wpack[:, i * 128:(i + 1) * 128]
                 for i, nm in enumerate(wnames)}
            w12 = wpack[:, 6 * 128:6 * 128 + 2]
            B = {"b%d" % (i + 1): bpack[:, i:i + 1] for i in range(3)}

            # inputs interleaved across the SP (HWDGE) and gpsimd (SWDGE)
            # queues so early graphs' data lands ASAP
            xT = slab.tile([128, npc], cdt, tag="xT")
            at = slab.tile([128, GPC * nk, sg], cdt, tag="at")
            nc.sync.dma_start(xT[:, 0:NCHUNK], xT_d[:, 0:NCHUNK])
            for g in range(GPC):
                gsl = slice(g * nk, (g + 1) * nk)
                nc.sync.dma_start(at[:, gsl, :], at_d[:, gsl, :])
                if g + 1 < nch:
                    c = g + 1
                    sl = slice(c * NCHUNK, (c + 1) * NCHUNK)
                    nc.sync.dma_start(xT[:, sl], xT_d[:, sl])

            def aggregate(nodemajor, name):
                """aggT[f, d] = sum_s x[s,f] * AT[s,d], per graph."""
                aggT = slab.tile([128, npc], cdt, name=name, tag=name)
                for g in range(GPC):
                    ps = ps_agg.tile([128, sg], f32)
                    for k, (off, klen) in enumerate(kts):
                        t = g * nk + k
                        nc.tensor.matmul(
                            ps[:],
                            mm(nodemajor[:klen, t * 128:t * 128 + 128]),
                            mm(at[:klen, t, :]),
                            start=(k == 0),
                            stop=(k == nk - 1),
                        )
                    nc.vector.tensor_copy(aggT[:, g * sg:(g + 1) * sg], ps[:])
                return aggT

            def dense2(wa, rhsa, wb, rhsb, bias, func, name):
                """out[f',n] = func(wa.T@rhsa + wb.T@rhsb + bias)."""
                out = slab.tile([128, npc], cdt, name=name, tag=name)
                for c in range(nch):
                    sl = slice(c * NCHUNK, (c + 1) * NCHUNK)
                    ps = ps_d.tile([128, NCHUNK], f32)
                    nc.tensor.matmul(ps[:], mm(wa[:]), mm(rhsa[:, sl]),
                                     start=True, stop=False)
                    nc.tensor.matmul(ps[:], mm(wb[:]), mm(rhsb[:, sl]),
                                     start=False, stop=True)
                    nc.scalar.activation(out[:, sl], ps[:], func, bias=bias[:])
                return out

            def to_nodemajor(featmajor, name):
                """Per-graph k-tiles: col block g*nk+k holds nodes
                [g*sg+off, g*sg+off+klen) in partitions [0, klen)."""
                out = slab.tile([128, GPC * nk * 128], cdt, name=name, tag=name)
                for g in range(GPC):
                    for k, (off, klen) in enumerate(kts):
                        t = g * nk + k
                        ps = ps_tp.tile([128, 128], cdt)
                        nc.tensor.transpose(
                            ps[:klen, :],
                            featmajor[:, g * sg + off:g * sg + off + klen],
                            ident[:],
                        )
                        nc.vector.tensor_copy(
                            out[:klen, t * 128:t * 128 + 128], ps[:klen, :]
                        )
                return out

            xN = to_nodemajor(xT, "xN")
            agg1 = aggregate(xN, "agg1")
            h1 = dense2(W["WL1"], agg1, W["WR1"], xT, B["b1"], AF.Relu, "h1")
            h1N = to_nodemajor(h1, "h1N")
            agg2 = aggregate(h1N, "agg2")
            h2 = dense2(W["WL2"], agg2, W["WR2"], h1, B["b2"], AF.Relu, "h2")
            hout = dense2(W["WLA"], h2, W["WLB"], h1, B["b3"], AF.Identity,
                          "hout")

            uvT = slab.tile([2, npc], f32, tag="uv")
            for c in range(nch):
                sl = slice(c * NCHUNK, (c + 1) * NCHUNK)
                ps = ps_uv.tile([2, NCHUNK], f32)
                nc.tensor.matmul(ps[:], mm(w12[:]), mm(hout[:, sl]),
                                 start=True, stop=True)
                nc.vector.tensor_copy(uvT[:, sl], ps[:])
                # split output DMAs per chunk so stores overlap compute;
                # gpsimd SWDGE keeps the SP queue free for input loads
                nc.gpsimd.dma_start(hout_d[:, sl], hout[:, sl])
                nc.gpsimd.dma_start(uv_d[:, sl], uvT[:, sl])

    _split_multi_waits(nc)
    return nc


USE_DT = "bf16"  # "bf16" | "f32" | "f32r"


def _np_cdt():
    if USE_DT == "bf16":
        import ml_dtypes

        return ml_dtypes.bfloat16
    return np.float32


class _Runner:
    """Compiled executor for one level-NEFF variant: builds the jitted
    shard_map body ONCE and reuses it across kernel() calls (mirrors
    concourse.bass2jax.run_bass_via_pjrt, which re-jits per call)."""

    def __init__(self, nc):
        import jax
        import concourse.mybir as mybir
        from concourse import bass2jax
        from jax.experimental.shard_map import shard_map
        from jax.sharding import Mesh, PartitionSpec

        bass2jax.install_neuronx_cc_hook()
        self._nc = nc
        in_names = []
        out_names = []
        out_avals = []
        zero_shapes = []
        partition_name = (
            nc.partition_id_tensor.name if nc.partition_id_tensor else None
        )
        for alloc in nc.m.functions[0].allocations:
            if not isinstance(alloc, mybir.MemoryLocationSet):
                continue
            name = alloc.memorylocations[0].name
            if alloc.kind == "ExternalInput":
                if name != partition_name:
                    in_names.append(name)
            elif alloc.kind == "ExternalOutput":
                shape = tuple(alloc.tensor_shape)
                dtype = mybir.dt.np(alloc.dtype)
                out_names.append(name)
                out_avals.append(jax.core.ShapedArray(shape, dtype))
                zero_shapes.append((shape, dtype))
        self.in_names = list(in_names)
        self.out_names = out_names
        self.out_avals = out_avals
        self.zero_shapes = zero_shapes
        n_params = len(in_names)
        n_outs = len(out_names)
        all_in = in_names + out_names
        if partition_name is not None:
            all_in.append(partition_name)

        def _body(*args):
            operands = list(args)
            if partition_name is not None:
                operands.append(bass2jax.partition_id_tensor())
            outs = bass2jax._bass_exec_p.bind(
                *operands,
                out_avals=tuple(out_avals),
                in_names=tuple(all_in),
                out_names=tuple(out_names),
                lowering_input_output_aliases=(),
                sim_require_finite=True,
                sim_require_nnan=True,
                nc=nc,
            )
            return tuple(outs)

        devices = jax.devices()[:NCORES]
        mesh = Mesh(np.asarray(devices), ("core",))
        in_specs = (PartitionSpec("core"),) * (n_params + n_outs)
        out_specs = (PartitionSpec("core"),) * n_outs
        donate = tuple(range(n_params, n_params + n_outs))
        self._fn = jax.jit(
            shard_map(_body, mesh=mesh, in_specs=in_specs,
                      out_specs=out_specs, check_rep=False),
            donate_argnums=donate,
            keep_unused=True,
        )

    def run(self, in_maps):
        concat_in = [
            np.concatenate([np.asarray(m[name]) for m in in_maps], axis=0)
            for name in self.in_names
        ]
        concat_zeros = [
            np.zeros((NCORES * s[0], *s[1:]), dt)
            for (s, dt) in self.zero_shapes
        ]
        out_arrs = self._fn(*concat_in, *concat_zeros)
        return [
            {
                name: np.asarray(out_arrs[i]).reshape(
                    NCORES, *self.out_avals[i].shape
                )[c]
                for i, name in enumerate(self.out_names)
            }
            for c in range(NCORES)
        ]


_build_lock = __import__("threading").Lock()
_building = {}


def _get_runner(sg):
    import threading

    key = (sg, USE_DT)
    with _build_lock:
        if key in _compiled:
            return _compiled[key]
        ev = _building.get(key)
        if ev is None:
            ev = threading.Event()
            _building[key] = ev
            is_builder = True
        else:
            is_builder = False
    if is_builder:
        try:
            r = _Runner(_build_level_nc(sg, USE_DT))
            with _build_lock:
                _compiled[key] = r
        finally:
            ev.set()
        return r
    ev.wait()
    return _compiled[(sg, USE_DT)]


def _prewarm_variants():
    """Compile the likely level-2/3 NEFF variants in the background while
    level 1 compiles/runs on the main thread."""
    import threading

    for sg in (256, 128):
        threading.Thread(
            target=lambda s=sg: _get_runner(s), daemon=True
        ).start()


def _get_level_nc(sg):
    return _get_runner(sg)._nc


def _run_level(Xslots, AThost, W, n, sg):
    """Xslots: [G, PADMAX, F] node-major; AThost: [G, sg, sg] normalized.
    W: level weight dict; n: [G] active counts (n <= sg).
    Returns Hout [G, sg, F], U [G, sg], V [G, sg] (all fp32)."""
    runner = _get_runner(sg)
    kts = _ktiles(sg)
    nk = len(kts)
    npc = GPC * sg
    cdt = _np_cdt()
    wpack = np.concatenate(
        [W["Wl1"].T, W["Wr1"].T, W["Wl2"].T, W["Wr2"].T,
         W["WlinA"], W["WlinB"], np.stack([W["w1"], W["w2"]], axis=1)],
        axis=1,
    ).astype(cdt)
    bpack = np.stack(
        [W["bl1"], W["bl2"], W["blin"]], axis=1
    ).astype(np.float32)
    shared = {
        "wpack": np.ascontiguousarray(wpack),
        "bpack": np.ascontiguousarray(bpack),
    }
    in_maps = []
    for c in range(NCORES):
        xs = Xslots[c * GPC:(c + 1) * GPC, :sg].reshape(npc, F).astype(cdt)
        xT = np.ascontiguousarray(xs.T)
        at = np.zeros((128, GPC * nk, sg), cdt)
        for g in range(GPC):
            for k, (off, klen) in enumerate(kts):
                t = g * nk + k
                at[:klen, t, :] = AThost[c * GPC + g, off:off + klen, :].astype(cdt)
        in_maps.append({"xT": xT, "AT": at, **shared})

    results = runner.run(in_maps)
    Hout = np.empty((G, sg, F), np.float32)
    U = np.empty((G, sg), np.float32)
    V = np.empty((G, sg), np.float32)
    for c in range(NCORES):
        h = results[c]["houtT"].astype(np.float32)   # [128, npc]
        uv = results[c]["uv"]                        # [2, npc]
        Hout[c * GPC:(c + 1) * GPC] = h.T.reshape(GPC, sg, F)
        U[c * GPC:(c + 1) * GPC] = uv[0].reshape(GPC, sg)
        V[c * GPC:(c + 1) * GPC] = uv[1].reshape(GPC, sg)
    return Hout, U, V


# ------------------------------------------------------------------ host ---

def _build_AT(edges, sg):
    AT = np.zeros((G, sg, sg), np.float32)
    for g in range(G):
        ls, ld = edges[g]
        if len(ls) == 0:
            continue
        cnt = np.bincount(ls.astype(np.int64) * sg + ld, minlength=sg * sg)
        cnt = cnt.reshape(sg, sg).astype(np.float32)
        indeg = np.bincount(ld, minlength=sg).astype(np.float32)
        AT[g] = cnt / np.maximum(indeg, 1.0)[None, :]
    return AT


def _greedy_match(ls, ld, order, n_g):
    """Sequential greedy matching in `order` == locally-dominant-edge
    rounds: an edge matches iff its rank is the minimum rank among
    surviving edges at BOTH endpoints. Returns indices of matched edges
    in rank order."""
    ne = len(order)
    if ne < 2500:
        # small graphs: plain sequential loop beats vectorized rounds
        rem = [True] * n_g
        ls_l = ls.tolist()
        ld_l = ld.tolist()
        win = []
        for idx in order.tolist():
            s = ls_l[idx]
            d = ld_l[idx]
            if rem[s] and rem[d]:
                rem[s] = False
                rem[d] = False
                win.append(idx)
        return np.asarray(win, np.int64)
    rank_of_edge = np.empty(ne, np.int64)
    rank_of_edge[order] = np.arange(ne)
    alive_n = np.ones(n_g, bool)
    matched = []
    idx = order.copy()  # surviving edges, any order; ranks give priority
    for _ in range(3):
        if len(idx) == 0:
            break
        best = np.full(n_g, ne, np.int64)
        r = rank_of_edge[idx]
        np.minimum.at(best, ls[idx], r)
        np.minimum.at(best, ld[idx], r)
        win = idx[(best[ls[idx]] == r) & (best[ld[idx]] == r)]
        if len(win) == 0:
            break
        matched.append(win)
        alive_n[ls[win]] = False
        alive_n[ld[win]] = False
        idx = idx[alive_n[ls[idx]] & alive_n[ld[idx]]]
    # finish the tail sequentially in rank order
    if len(idx):
        idx = idx[np.argsort(rank_of_edge[idx])]
        rem = alive_n.tolist()
        ls_l = ls[idx].tolist()
        ld_l = ld[idx].tolist()
        tail = []
        for j, (s, d_) in enumerate(zip(ls_l, ld_l)):
            if rem[s] and rem[d_]:
                rem[s] = False
                rem[d_] = False
                tail.append(idx[j])
        if tail:
            matched.append(np.asarray(tail, np.int64))
    if not matched:
        return np.empty(0, np.int64)
    w = np.concatenate(matched)
    return w[np.argsort(rank_of_edge[w])]


def _pool_graph(Hout_g, u_g, v_g, ls, ld, n_g, bp):
    raw = u_g[ls] + v_g[ld] + np.float32(bp)
    m = np.full(n_g, -np.inf, np.float32)
    np.maximum.at(m, ld, raw)
    e = np.exp(raw - m[ld], dtype=np.float32)
    z = np.bincount(ld, weights=e, minlength=n_g).astype(np.float32)
    score = e / z[ld] + np.float32(0.5)

    order = np.argsort(-score, kind="stable")
    win = _greedy_match(ls, ld, order, n_g)
    cnt = len(win)
    cluster = np.full(n_g, -1, np.int64)
    cluster[ls[win]] = np.arange(cnt)
    cluster[ld[win]] = np.arange(cnt)
    cscores = score[win]
    rem_nodes = np.flatnonzero(cluster < 0)
    cluster[rem_nodes] = cnt + np.arange(len(rem_nodes))
    n_new = cnt + len(rem_nodes)

    csc = np.concatenate(
        [cscores.astype(np.float32), np.ones(len(rem_nodes), np.float32)]
    )
    newX = np.zeros((PADMAX, F), np.float32)
    np.add.at(newX, cluster, Hout_g[:n_g])
    newX[:n_new] *= csc[:, None]

    keys = np.unique(cluster[ls] * PADMAX + cluster[ld])
    return newX, n_new, (
        (keys // PADMAX).astype(np.int32),
        (keys % PADMAX).astype(np.int32),
    )


def _level_weights(params, i):
    Wlin = params["Wlin%d" % i]
    Wp = params["Wp%d" % i]
    return {
        "Wl1": params["W%dl" % (2 * i - 1)],
        "bl1": params["b%dl" % (2 * i - 1)],
        "Wr1": params["W%dr" % (2 * i - 1)],
        "Wl2": params["W%dl" % (2 * i)],
        "bl2": params["b%dl" % (2 * i)],
        "Wr2": params["W%dr" % (2 * i)],
        "WlinA": np.ascontiguousarray(Wlin[:, :H].T),
        "WlinB": np.ascontiguousarray(Wlin[:, H:].T),
        "blin": params["blin%d" % i],
        "w1": np.ascontiguousarray(Wp[0, :H]),
        "w2": np.ascontiguousarray(Wp[0, H:]),
        "bp": float(np.asarray(params["bp%d" % i]).reshape(-1)[0]),
    }


def _log_softmax(x):
    m = x.max(axis=1, keepdims=True)
    e = np.exp(x - m)
    return (x - m) - np.log(e.sum(axis=1, keepdims=True))


def _pick_sg(nmax):
    for sg in (128, 256, 320):
        if nmax <= sg:
            return sg
    raise AssertionError("graph size %d exceeds 320" % nmax)


def kernel(x, edge_index, batch, params):
    x = np.asarray(x, np.float32)
    params = {k: np.asarray(v, np.float32) for k, v in params.items()}
    src = np.asarray(edge_index[0], np.int64)
    dst = np.asarray(edge_index[1], np.int64)

    g_of_edge = src // NPG
    assert (dst // NPG == g_of_edge).all(), (
        "kernel assumes within-graph edges (dst = (src//NPG)*NPG + off)"
    )
    order = np.argsort(g_of_edge, kind="stable")
    bounds = np.searchsorted(g_of_edge[order], np.arange(G + 1))
    edges = []
    for g in range(G):
        sel = order[bounds[g]: bounds[g + 1]]
        edges.append(((src[sel] - g * NPG).astype(np.int32),
                      (dst[sel] - g * NPG).astype(np.int32)))

    X = np.zeros((G, PADMAX, F), np.float32)
    X[:, :NPG] = x.reshape(G, NPG, F)
    n = np.full(G, NPG, np.int64)

    total_sum = np.zeros((G, F), np.float32)
    total_cnt = np.zeros(G, np.int64)

    for i in (1, 2, 3):
        W = _level_weights(params, i)
        sg = _pick_sg(int(n.max()))
        AT = _build_AT(edges, sg)
        Hout, U, V = _run_level(X, AT, W, n, sg)
        newX = np.zeros((G, PADMAX, F), np.float32)
        new_n = np.empty(G, np.int64)
        new_edges = []
        for g in range(G):
            total_sum[g] += Hout[g, : n[g]].sum(axis=0)
            total_cnt[g] += n[g]
            nx, nn, ne = _pool_graph(
                Hout[g], U[g], V[g], edges[g][0], edges[g][1], n[g], W["bp"]
            )
            newX[g] = nx
            new_n[g] = nn
            new_edges.append(ne)
        X, n, edges = newX, new_n, new_edges

    for g in range(G):
        total_sum[g] += X[g, : n[g]].sum(axis=0)
        total_cnt[g] += n[g]

    gv = total_sum / np.maximum(total_cnt, 1)[:, None].astype(np.float32)
    g1 = np.maximum(gv @ params["Wfc1"].T + params["bfc1"], 0.0)
    out = g1 @ params["Wfc2"].T + params["bfc2"]
    return _log_softmax(out).astype(np.float32)


# revision 45
# speedup vs baseline: 1.0314x; 1.0314x over previous
"""Trainium2 Bass kernel for nn_GCNDiehlq1 (SAGEConv x2 + lin + EdgePooling, x3
levels, mean readout + MLP + log_softmax).

Structure (validated against the jax reference at ~1e-7 rel err):
- All edges are within-graph, so SAGE mean aggregation == per-graph dense
  A_norm @ x with AT[s,d] = count(s->d)/max(indeg(d),1). The 64 graphs are
  sharded 8-per-core across 8 NeuronCores.
- The device kernel computes one level (2 SAGE convs + lin + edge-score
  projections u,v) for 8 graphs in feature-major layout. The builder is
  parameterized by slots-per-graph (320 for level 1; levels 2/3 pick
  256/128 variants based on actual post-pool graph sizes) -> one NEFF per
  size variant, reused across calls.
- Host does the inherently sequential EdgePooling (per-dst softmax from
  u,v, stable sort, greedy merge scan, coalesce) and the final readout.
  Per-graph compact cluster relabeling is strictly order-preserving
  w.r.t. the reference's global labels within each graph, so coalesce
  order and sort tie-breaks match the reference exactly.
"""

import numpy as np

N = 20480
NPG = 320
G = 64
F = 128
H = 128
C = 6
PADMAX = 384                 # host-side cluster-id padding (>= 320)
NCORES = 8
GPC = G // NCORES            # 8 graphs per core
NCHUNK = 512                 # dense matmul moving free dim

_compiled = {}


def _ktiles(sg):
    """K-tile (offset, length) list for sg slots per graph."""
    out = []
    off = 0
    while off < sg:
        out.append((off, min(128, sg - off)))
        off += 128
    return out


# ---------------------------------------------------------------- device ---

def _apply_tile_patch():
    """This walrus build rejects >1 sem waits on TPB_CTRL (Drain/NoOp):
    'Too many sync wait commands'. Split the TileContext exit-barrier waits
    across one NOP per logical proc, then emit the drain bare."""
    import concourse.tile as tile
    from concourse.vector_clock import ScopedClock, VectorClock

    if getattr(tile.TileContext, "_drain_patched", False):
        return

    def _patched(self, tick_clock, wait_clock):
        full = tick_clock.global_clock
        nprocs = len(full)
        for proc in range(nprocs):
            tick = full[proc]
            if tick <= 0:
                continue
            vec = [0] * nprocs
            vec[proc] = tick
            nop_inst = self.nc.sync.nop(nofuse=True, hint="pre_drain_%d" % proc)
            wait_clock.add_sem_waits(
                nop_inst.ins, ScopedClock({None: VectorClock(vec)})
            )
        self.nc.sync.drain()
        if getattr(type(self), "_keep_tail_barrier", True):
            self.nc.all_engine_barrier()
        assert self.sems is not None
        popped = self.nc._tile_sem_poison_stack.pop()
        assert popped is self._sem_poison
        if getattr(type(self), "_keep_sem_reset", True):
            self.nc.clear_and_free_semaphores(
                list(self.sems.allocated().values()))
            self.nc.all_engine_barrier()
        else:
            # skip the end-of-NEFF sem clear + second barrier; only do the
            # python-side free-pool bookkeeping
            sem_nums = [s.num for s in self.sems.allocated().values()]
            self.nc._state.prepend_free_semaphores(sem_nums)
            for poison_set in self.nc._tile_sem_poison_stack:
                poison_set.update(sem_nums)

    tile.TileContext._drain_and_barrier = _patched
    tile.TileContext._drain_patched = True
    # re-execution works without the end-of-NEFF sem clear: the runtime
    # reinitializes semaphore state per execution (verified empirically,
    # two back-to-back kernel() calls bitwise identical)
    tile.TileContext._keep_sem_reset = False
    tile.TileContext._keep_tail_barrier = False


def _split_multi_waits(nc):
    """This walrus build allows at most ONE sync wait per instruction.
    Insert single-wait NoOps (same engine, just before) for the extras."""
    import concourse.mybir as mybir

    for f in nc.m.functions:
        for bb in f.blocks:
            insts = list(bb.instructions)
            out = []
            changed = False
            for ins in insts:
                si = ins.sync_info
                if si is not None and len(si.on_wait) > 1:
                    waits = list(si.on_wait)
                    for j, w in enumerate(waits[:-1]):
                        nop = mybir.InstNoOp(name="%s_w%d" % (ins.name, j))
                        nop.engine = ins.engine
                        nop.sync_info = mybir.SyncInfo(on_wait=[w], on_update=[])
                        out.append(nop)
                    ins.sync_info = mybir.SyncInfo(
                        on_wait=[waits[-1]], on_update=list(si.on_update)
                    )
                    changed = True
                out.append(ins)
            if changed:
                bb.instructions = out


def _build_level_nc(sg, use_bf16):
    """One level for 8 graphs with sg slots per graph, feature-major
    activations [128, 8*sg]."""
    import concourse.bass as bass
    import concourse.mybir as mybir
    import concourse.tile as tile
    from concourse.masks import make_identity

    _apply_tile_patch()
    f32 = mybir.dt.float32
    f32r_mm = use_bf16 == "f32r"
    cdt = mybir.dt.bfloat16 if use_bf16 == "bf16" else f32
    mdt = mybir.dt.float32r if f32r_mm else cdt

    def mm(x):
        # reinterpret fp32 operands as float32r at matmul sites only
        return x.bitcast(mdt) if f32r_mm else x

    AF = mybir.ActivationFunctionType

    kts = _ktiles(sg)
    nk = len(kts)
    npc = GPC * sg
    nch = (npc + NCHUNK - 1) // NCHUNK
    assert npc % NCHUNK == 0

    nc = bass.Bass("TRN2", target_bir_lowering=False,
                   disable_frame_to_traceback=True)
    xT_d = nc.declare_dram_parameter("xT", [128, npc], cdt, isOutput=False)
    at_d = nc.declare_dram_parameter("AT", [128, GPC * nk, sg], cdt,
                                     isOutput=False)
    # all six [128,128] weights + w12 packed into one [128, 770] slab
    wpack_d = nc.declare_dram_parameter("wpack", [128, 6 * 128 + 2], cdt,
                                        isOutput=False)
    bpack_d = nc.declare_dram_parameter("bpack", [128, 3], f32, isOutput=False)
    hout_d = nc.declare_dram_parameter("houtT", [128, npc], cdt, isOutput=True)
    uv_d = nc.declare_dram_parameter("uv", [2, npc], f32, isOutput=True)

    with tile.TileContext(nc) as tc:
        with (
            tc.tile_pool(name="slab", bufs=1) as slab,
            tc.tile_pool(name="wts", bufs=1) as wts,
            tc.tile_pool(name="ps_agg", bufs=2, space="PSUM") as ps_agg,
            tc.tile_pool(name="ps_d", bufs=2, space="PSUM") as ps_d,
            tc.tile_pool(name="ps_tp", bufs=2, space="PSUM") as ps_tp,
        ):
            ident = wts.tile([128, 128], cdt)
            make_identity(nc, ident[:])

            # weights via gpsimd (SWDGE) to keep the SP queue free for inputs
            wpack = wts.tile([128, 6 * 128 + 2], cdt)
            nc.gpsimd.dma_start(wpack[:], wpack_d[:])
            bpack = wts.tile([128, 3], f32)
            nc.gpsimd.dma_start(bpack[:], bpack_d[:])
            wnames = ("WL1", "WR1", "WL2", "WR2", "WLA", "WLB")
            W = {nm: wpack[:, i * 128:(i + 1) * 128]
                 for i, nm in enumerate(wnames)}
            w12 = wpack[:, 6 * 128:6 * 128 + 2]
            B = {"b%d" % (i + 1): bpack[:, i:i + 1] for i in range(3)}

            # inputs interleaved across the SP (HWDGE) and gpsimd (SWDGE)
            # queues so early graphs' data lands ASAP
            xT = slab.tile([128, npc], cdt, tag="xT")
            at = slab.tile([128, GPC * nk, sg], cdt, tag="at")
            nc.sync.dma_start(xT[:, 0:NCHUNK], xT_d[:, 0:NCHUNK])
            for g in range(GPC):
                gsl = slice(g * nk, (g + 1) * nk)
                nc.sync.dma_start(at[:, gsl, :], at_d[:, gsl, :])
                if g + 1 < nch:
                    c = g + 1
                    sl = slice(c * NCHUNK, (c + 1) * NCHUNK)
                    nc.sync.dma_start(xT[:, sl], xT_d[:, sl])

            def aggregate(nodemajor, name):
                """aggT[f, d] = sum_s x[s,f] * AT[s,d], per graph."""
                aggT = slab.tile([128, npc], cdt, name=name, tag=name)
                for g in range(GPC):
                    ps = ps_agg.tile([128, sg], f32)
                    for k, (off, klen) in enumerate(kts):
                        t = g * nk + k
                        nc.tensor.matmul(
                            ps[:],
                            mm(nodemajor[:klen, t * 128:t * 128 + 128]),
                            mm(at[:klen, t, :]),
                            start=(k == 0),
                            stop=(k == nk - 1),
                        )
                    nc.vector.tensor_copy(aggT[:, g * sg:(g + 1) * sg], ps[:])
                return aggT

            def dense2(wa, rhsa, wb, rhsb, bias, func, name):
                """out[f',n] = func(wa.T@rhsa + wb.T@rhsb + bias)."""
                out = slab.tile([128, npc], cdt, name=name, tag=name)
                for c in range(nch):
                    sl = slice(c * NCHUNK, (c + 1) * NCHUNK)
                    ps = ps_d.tile([128, NCHUNK], f32)
                    nc.tensor.matmul(ps[:], mm(wa[:]), mm(rhsa[:, sl]),
                                     start=True, stop=False)
                    nc.tensor.matmul(ps[:], mm(wb[:]), mm(rhsb[:, sl]),
                                     start=False, stop=True)
                    nc.scalar.activation(out[:, sl], ps[:], func, bias=bias[:])
                return out

            def to_nodemajor(featmajor, name):
                """Per-graph k-tiles: col block g*nk+k holds nodes
                [g*sg+off, g*sg+off+klen) in partitions [0, klen)."""
                out = slab.tile([128, GPC * nk * 128], cdt, name=name, tag=name)
                for g in range(GPC):
                    for k, (off, klen) in enumerate(kts):
                        t = g * nk + k
                        ps = ps_tp.tile([128, 128], cdt)
                        nc.tensor.transpose(
                            ps[:klen, :],
                            featmajor[:, g * sg + off:g * sg + off + klen],
                            ident[:],
                        )
                        nc.vector.tensor_copy(
                            out[:klen, t * 128:t * 128 + 128], ps[:klen, :]
                        )
                return out

            xN = to_nodemajor(xT, "xN")
            agg1 = aggregate(xN, "agg1")
            h1 = dense2(W["WL1"], agg1, W["WR1"], xT, B["b1"], AF.Relu, "h1")
            h1N = to_nodemajor(h1, "h1N")
            agg2 = aggregate(h1N, "agg2")
            h2 = dense2(W["WL2"], agg2, W["WR2"], h1, B["b2"], AF.Relu, "h2")
            hout = dense2(W["WLA"], h2, W["WLB"], h1, B["b3"], AF.Identity,
                          "hout")

            uvT = slab.tile([2, npc], f32, tag="uv")
            for c in range(nch):
                sl = slice(c * NCHUNK, (c + 1) * NCHUNK)
                ps = ps_tp.tile([2, NCHUNK], f32, tag="uvps")
                nc.tensor.matmul(ps[:], mm(w12[:]), mm(hout[:, sl]),
                                 start=True, stop=True)
                nc.vector.tensor_copy(uvT[:, sl], ps[:])
                # split output DMAs per chunk so stores overlap compute;
                # gpsimd SWDGE keeps the SP queue free for input loads
                nc.gpsimd.dma_start(hout_d[:, sl], hout[:, sl])
                nc.gpsimd.dma_start(uv_d[:, sl], uvT[:, sl])

    _split_multi_waits(nc)
    return nc


USE_DT = "bf16"  # "bf16" | "f32" | "f32r"


def _np_cdt():
    if USE_DT == "bf16":
        import ml_dtypes

        return ml_dtypes.bfloat16
    return np.float32


class _Runner:
    """Compiled executor for one level-NEFF variant: builds the jitted
    shard_map body ONCE and reuses it across kernel() calls (mirrors
    concourse.bass2jax.run_bass_via_pjrt, which re-jits per call)."""

    def __init__(self, nc):
        import jax
        import concourse.mybir as mybir
        from concourse import bass2jax
        from jax.experimental.shard_map import shard_map
        from jax.sharding import Mesh, PartitionSpec

        bass2jax.install_neuronx_cc_hook()
        self._nc = nc
        in_names = []
        out_names = []
        out_avals = []
        zero_shapes = []
        partition_name = (
            nc.partition_id_tensor.name if nc.partition_id_tensor else None
        )
        for alloc in nc.m.functions[0].allocations:
            if not isinstance(alloc, mybir.MemoryLocationSet):
                continue
            name = alloc.memorylocations[0].name
            if alloc.kind == "ExternalInput":
                if name != partition_name:
                    in_names.append(name)
            elif alloc.kind == "ExternalOutput":
                shape = tuple(alloc.tensor_shape)
                dtype = mybir.dt.np(alloc.dtype)
                out_names.append(name)
                out_avals.append(jax.core.ShapedArray(shape, dtype))
                zero_shapes.append((shape, dtype))
        self.in_names = list(in_names)
        self.out_names = out_names
        self.out_avals = out_avals
        self.zero_shapes = zero_shapes
        n_params = len(in_names)
        n_outs = len(out_names)
        all_in = in_names + out_names
        if partition_name is not None:
            all_in.append(partition_name)

        def _body(*args):
            operands = list(args)
            if partition_name is not None:
                operands.append(bass2jax.partition_id_tensor())
            outs = bass2jax._bass_exec_p.bind(
                *operands,
                out_avals=tuple(out_avals),
                in_names=tuple(all_in),
                out_names=tuple(out_names),
                lowering_input_output_aliases=(),
                sim_require_finite=True,
                sim_require_nnan=True,
                nc=nc,
            )
            return tuple(outs)

        devices = jax.devices()[:NCORES]
        mesh = Mesh(np.asarray(devices), ("core",))
        in_specs = (PartitionSpec("core"),) * (n_params + n_outs)
        out_specs = (PartitionSpec("core"),) * n_outs
        donate = tuple(range(n_params, n_params + n_outs))
        self._fn = jax.jit(
            shard_map(_body, mesh=mesh, in_specs=in_specs,
                      out_specs=out_specs, check_rep=False),
            donate_argnums=donate,
            keep_unused=True,
        )

    def run(self, in_maps):
        concat_in = [
            np.concatenate([np.asarray(m[name]) for m in in_maps], axis=0)
            for name in self.in_names
        ]
        concat_zeros = [
            np.zeros((NCORES * s[0], *s[1:]), dt)
            for (s, dt) in self.zero_shapes
        ]
        out_arrs = self._fn(*concat_in, *concat_zeros)
        return [
            {
                name: np.asarray(out_arrs[i]).reshape(
                    NCORES, *self.out_avals[i].shape
                )[c]
                for i, name in enumerate(self.out_names)
            }
            for c in range(NCORES)
        ]


_build_lock = __import__("threading").Lock()
_building = {}


def _get_runner(sg):
    import threading

    key = (sg, USE_DT)
    with _build_lock:
        if key in _compiled:
            return _compiled[key]
        ev = _building.get(key)
        if ev is None:
            ev = threading.Event()
            _building[key] = ev
            is_builder = True
        else:
            is_builder = False
    if is_builder:
        try:
            r = _Runner(_build_level_nc(sg, USE_DT))
            with _build_lock:
                _compiled[key] = r
        finally:
            ev.set()
        return r
    ev.wait()
    return _compiled[(sg, USE_DT)]


def _prewarm_variants():
    """Compile the likely level-2/3 NEFF variants in the background while
    level 1 compiles/runs on the main thread."""
    import threading

    for sg in (256, 128):
        threading.Thread(
            target=lambda s=sg: _get_runner(s), daemon=True
        ).start()


def _get_level_nc(sg):
    return _get_runner(sg)._nc


def _run_level(Xslots, AThost, W, n, sg):
    """Xslots: [G, PADMAX, F] node-major; AThost: [G, sg, sg] normalized.
    W: level weight dict; n: [G] active counts (n <= sg).
    Returns Hout [G, sg, F], U [G, sg], V [G, sg] (all fp32)."""
    runner = _get_runner(sg)
    kts = _ktiles(sg)
    nk = len(kts)
    npc = GPC * sg
    cdt = _np_cdt()
    wpack = np.concatenate(
        [W["Wl1"].T, W["Wr1"].T, W["Wl2"].T, W["Wr2"].T,
         W["WlinA"], W["WlinB"], np.stack([W["w1"], W["w2"]], axis=1)],
        axis=1,
    ).astype(cdt)
    bpack = np.stack(
        [W["bl1"], W["bl2"], W["blin"]], axis=1
    ).astype(np.float32)
    shared = {
        "wpack": np.ascontiguousarray(wpack),
        "bpack": np.ascontiguousarray(bpack),
    }
    in_maps = []
    for c in range(NCORES):
        xs = Xslots[c * GPC:(c + 1) * GPC, :sg].reshape(npc, F).astype(cdt)
        xT = np.ascontiguousarray(xs.T)
        at = np.zeros((128, GPC * nk, sg), cdt)
        for g in range(GPC):
            for k, (off, klen) in enumerate(kts):
                t = g * nk + k
                at[:klen, t, :] = AThost[c * GPC + g, off:off + klen, :].astype(cdt)
        in_maps.append({"xT": xT, "AT": at, **shared})

    results = runner.run(in_maps)
    Hout = np.empty((G, sg, F), np.float32)
    U = np.empty((G, sg), np.float32)
    V = np.empty((G, sg), np.float32)
    for c in range(NCORES):
        h = results[c]["houtT"].astype(np.float32)   # [128, npc]
        uv = results[c]["uv"]                        # [2, npc]
        Hout[c * GPC:(c + 1) * GPC] = h.T.reshape(GPC, sg, F)
        U[c * GPC:(c + 1) * GPC] = uv[0].reshape(GPC, sg)
        V[c * GPC:(c + 1) * GPC] = uv[1].reshape(GPC, sg)
    return Hout, U, V


# ------------------------------------------------------------------ host ---

def _build_AT(edges, sg):
    AT = np.zeros((G, sg, sg), np.float32)
    for g in range(G):
        ls, ld = edges[g]
        if len(ls) == 0:
            continue
        cnt = np.bincount(ls.astype(np.int64) * sg + ld, minlength=sg * sg)
        cnt = cnt.reshape(sg, sg).astype(np.float32)
        indeg = np.bincount(ld, minlength=sg).astype(np.float32)
        AT[g] = cnt / np.maximum(indeg, 1.0)[None, :]
    return AT


def _greedy_match(ls, ld, order, n_g):
    """Sequential greedy matching in `order` == locally-dominant-edge
    rounds: an edge matches iff its rank is the minimum rank among
    surviving edges at BOTH endpoints. Returns indices of matched edges
    in rank order."""
    ne = len(order)
    if ne < 2500:
        # small graphs: plain sequential loop beats vectorized rounds
        rem = [True] * n_g
        ls_l = ls.tolist()
        ld_l = ld.tolist()
        win = []
        for idx in order.tolist():
            s = ls_l[idx]
            d = ld_l[idx]
            if rem[s] and rem[d]:
                rem[s] = False
                rem[d] = False
                win.append(idx)
        return np.asarray(win, np.int64)
    rank_of_edge = np.empty(ne, np.int64)
    rank_of_edge[order] = np.arange(ne)
    alive_n = np.ones(n_g, bool)
    matched = []
    idx = order.copy()  # surviving edges, any order; ranks give priority
    for _ in range(3):
        if len(idx) == 0:
            break
        best = np.full(n_g, ne, np.int64)
        r = rank_of_edge[idx]
        np.minimum.at(best, ls[idx], r)
        np.minimum.at(best, ld[idx], r)
        win = idx[(best[ls[idx]] == r) & (best[ld[idx]] == r)]
        if len(win) == 0:
            break
        matched.append(win)
        alive_n[ls[win]] = False
        alive_n[ld[win]] = False
        idx = idx[alive_n[ls[idx]] & alive_n[ld[idx]]]
    # finish the tail sequentially in rank order
    if len(idx):
        idx = idx[np.argsort(rank_of_edge[idx])]
        rem = alive_n.tolist()
        ls_l = ls[idx].tolist()
        ld_l = ld[idx].tolist()
        tail = []
        for j, (s, d_) in enumerate(zip(ls_l, ld_l)):
            if rem[s] and rem[d_]:
                rem[s] = False
                rem[d_] = False
                tail.append(idx[j])
        if tail:
            matched.append(np.asarray(tail, np.int64))
    if not matched:
        return np.empty(0, np.int64)
    w = np.concatenate(matched)
    return w[np.argsort(rank_of_edge[w])]


def _pool_graph(Hout_g, u_g, v_g, ls, ld, n_g, bp):
    raw = u_g[ls] + v_g[ld] + np.float32(bp)
    m = np.full(n_g, -np.inf, np.float32)
    np.maximum.at(m, ld, raw)
    e = np.exp(raw - m[ld], dtype=np.float32)
    z = np.bincount(ld, weights=e, minlength=n_g).astype(np.float32)
    score = e / z[ld] + np.float32(0.5)

    order = np.argsort(-score, kind="stable")
    win = _greedy_match(ls, ld, order, n_g)
    cnt = len(win)
    cluster = np.full(n_g, -1, np.int64)
    cluster[ls[win]] = np.arange(cnt)
    cluster[ld[win]] = np.arange(cnt)
    cscores = score[win]
    rem_nodes = np.flatnonzero(cluster < 0)
    cluster[rem_nodes] = cnt + np.arange(len(rem_nodes))
    n_new = cnt + len(rem_nodes)

    csc = np.concatenate(
        [cscores.astype(np.float32), np.ones(len(rem_nodes), np.float32)]
    )
    newX = np.zeros((PADMAX, F), np.float32)
    np.add.at(newX, cluster, Hout_g[:n_g])
    newX[:n_new] *= csc[:, None]

    keys = np.unique(cluster[ls] * PADMAX + cluster[ld])
    return newX, n_new, (
        (keys // PADMAX).astype(np.int32),
        (keys % PADMAX).astype(np.int32),
    )


def _level_weights(params, i):
    Wlin = params["Wlin%d" % i]
    Wp = params["Wp%d" % i]
    return {
        "Wl1": params["W%dl" % (2 * i - 1)],
        "bl1": params["b%dl" % (2 * i - 1)],
        "Wr1": params["W%dr" % (2 * i - 1)],
        "Wl2": params["W%dl" % (2 * i)],
        "bl2": params["b%dl" % (2 * i)],
        "Wr2": params["W%dr" % (2 * i)],
        "WlinA": np.ascontiguousarray(Wlin[:, :H].T),
        "WlinB": np.ascontiguousarray(Wlin[:, H:].T),
        "blin": params["blin%d" % i],
        "w1": np.ascontiguousarray(Wp[0, :H]),
        "w2": np.ascontiguousarray(Wp[0, H:]),
        "bp": float(np.asarray(params["bp%d" % i]).reshape(-1)[0]),
    }


def _log_softmax(x):
    m = x.max(axis=1, keepdims=True)
    e = np.exp(x - m)
    return (x - m) - np.log(e.sum(axis=1, keepdims=True))


def _pick_sg(nmax):
    for sg in (128, 256, 320):
        if nmax <= sg:
            return sg
    raise AssertionError("graph size %d exceeds 320" % nmax)


def kernel(x, edge_index, batch, params):
    x = np.asarray(x, np.float32)
    params = {k: np.asarray(v, np.float32) for k, v in params.items()}
    src = np.asarray(edge_index[0], np.int64)
    dst = np.asarray(edge_index[1], np.int64)

    g_of_edge = src // NPG
    assert (dst // NPG == g_of_edge).all(), (
        "kernel assumes within-graph edges (dst = (src//NPG)*NPG + off)"
    )
    order = np.argsort(g_of_edge, kind="stable")
    bounds = np.searchsorted(g_of_edge[order], np.arange(G + 1))
    edges = []
    for g in range(G):
        sel = order[bounds[g]: bounds[g + 1]]
        edges.append(((src[sel] - g * NPG).astype(np.int32),
                      (dst[sel] - g * NPG).astype(np.int32)))

    X = np.zeros((G, PADMAX, F), np.float32)
    X[:, :NPG] = x.reshape(G, NPG, F)
    n = np.full(G, NPG, np.int64)

    total_sum = np.zeros((G, F), np.float32)
    total_cnt = np.zeros(G, np.int64)

    for i in (1, 2, 3):
        W = _level_weights(params, i)
        sg = _pick_sg(int(n.max()))
        AT = _build_AT(edges, sg)
        Hout, U, V = _run_level(X, AT, W, n, sg)
        newX = np.zeros((G, PADMAX, F), np.float32)
        new_n = np.empty(G, np.int64)
        new_edges = []
        for g in range(G):
            total_sum[g] += Hout[g, : n[g]].sum(axis=0)
            total_cnt[g] += n[g]
            nx, nn, ne = _pool_graph(
                Hout[g], U[g], V[g], edges[g][0], edges[g][1], n[g], W["bp"]
            )
            newX[g] = nx
            new_n[g] = nn
            new_edges.append(ne)
        X, n, edges = newX, new_n, new_edges

    for g in range(G):
        total_sum[g] += X[g, : n[g]].sum(axis=0)
        total_cnt[g] += n[g]

    gv = total_sum / np.maximum(total_cnt, 1)[:, None].astype(np.float32)
    g1 = np.maximum(gv @ params["Wfc1"].T + params["bfc1"], 0.0)
    out = g1 @ params["Wfc2"].T + params["bfc2"]
    return _log_softmax(out).astype(np.float32)
